# revision 13
# baseline (speedup 1.0000x reference)
"""LowBitEncoder Trainium2 kernel.

y = LayerNorm((x @ tern(W).T + bias) * scale) -> tanh(y/qs) -> round-to-1/127 grid.

Wall-clock of kernel() is dominated by the axon host<->device tunnel
(~67 MB/s h2d, ~37 MB/s d2h), so the design minimizes shipped bytes.

Dense path (general):
  - x shipped as fp16 [16384, 4096] (128 MiB), batch-sharded over 8 cores,
    placed with per-device async device_put (parallel streams).
  - weight ternarized AND base-27 packed on host (4 values/byte,
    [DIN, DOUT/4] i8, 4 MiB), sent once per device via a replicated spec
    (no 8x host copies); decoded on-device to an fp16 W^T DRAM scratch
    with round(r/base) magic-number arithmetic (exact for ternary).
  - per 512-token block: DMA-transpose x^T tiles; stream W^T quarter-slabs;
    fp16 matmuls accumulate y[128t, 4096o] f32 in 8 PSUM banks; DVE evac
    (+row sums), ACT square (+row sumsq), LN normalize, ACT tanh(1/qs),
    round via magic trick, int8 out.
  - output returned as int8 grid indices (64 MiB); the d2h fetch is
    pipelined with the int8 -> f32/127 conversion shard by shard.
  - output donation buffers are created on-device (jnp.zeros), not shipped.

Sparse path (trivial LN params and <=16 active rows/cols of tern(W), which
BitNet-style U(-0.1,0.1) init with threshold 0.1 almost always yields):
ships only the active x columns pre-transposed (f32, ~1 MiB), computes the
active-column matmul + exact LayerNorm statistics + tanh/round on device,
and returns per-token active outputs plus the shared inactive-column value;
the host broadcast-fills the full [B,S,DOUT] f32 output.
"""
import numpy as np
from contextlib import ExitStack
from functools import partial
from concurrent.futures import ThreadPoolExecutor

import jax
import jax.numpy as jnp
from jax.sharding import Mesh, PartitionSpec, NamedSharding
from jax.experimental.shard_map import shard_map

import concourse.bass as bass
from concourse import bacc
import concourse.tile as tile
import concourse.mybir as mybir
from concourse import bass2jax

B, S, DIN, DOUT = 8, 2048, 4096, 4096
P = 128
T = S                 # tokens per core (batch-sharded)
NCORES = 8
THRESH = 0.1
LN_EPS = 1e-5
MAGIC = 12582912.0    # 1.5 * 2**23: round-half-even for |v| < 2**22
f32, f16, i8 = mybir.dt.float32, mybir.dt.float16, mybir.dt.int8
Alu = mybir.AluOpType
Act = mybir.ActivationFunctionType

_CACHE = {}


def _build(trivial_params: bool):
    """Build the Bass program. trivial_params: bias==0, scale==1, gamma==1, beta==0."""
    T_B = 512 if trivial_params else 256       # tokens per block
    NBLK = T // T_B
    NTT = T_B // P                             # 4 t-tiles per block
    KT = DIN // P                              # 32 k-tiles
    NOP = 4                                    # o-quarter count
    OPW = DOUT // NOP                          # 1024 columns per quarter
    NOS = OPW // 512                           # 2 o-slices of 512 per quarter

    nc = bacc.Bacc("TRN2", target_bir_lowering=False, debug=False)
    x_d = nc.dram_tensor("x", [T, DIN], f16, kind="ExternalInput")
    wp_d = nc.dram_tensor("wp", [DIN, DOUT // 4], i8, kind="ExternalInput")
    bias_d = nc.dram_tensor("bias", [DOUT], f32, kind="ExternalInput")
    scale_d = nc.dram_tensor("scale", [DOUT], f32, kind="ExternalInput")
    gam_d = nc.dram_tensor("gam", [DOUT], f32, kind="ExternalInput")
    bet_d = nc.dram_tensor("bet", [DOUT], f32, kind="ExternalInput")
    qs_d = nc.dram_tensor("qs", [1], f32, kind="ExternalInput")
    out_d = nc.dram_tensor("out", [T, DOUT], i8, kind="ExternalOutput")
    wt_h = nc.dram_tensor("wt_h", [DIN, DOUT], f16)   # fp16 W^T scratch

    with tile.TileContext(nc) as tc:
        with ExitStack() as ctx:
            consts = ctx.enter_context(tc.tile_pool(name="consts", bufs=1))
            wprep = ctx.enter_context(tc.tile_pool(name="wprep", bufs=2))
            xt_pool = ctx.enter_context(tc.tile_pool(name="xt", bufs=2))
            wst = ctx.enter_context(tc.tile_pool(name="wst", bufs=3))
            ypool = ctx.enter_context(tc.tile_pool(name="y", bufs=NTT))
            opool = ctx.enter_context(tc.tile_pool(name="o", bufs=2))
            stat = ctx.enter_context(tc.tile_pool(name="stat", bufs=2 * NTT + 2))
            sq_pool = ctx.enter_context(tc.tile_pool(name="sq", bufs=2))
            pp = ctx.enter_context(tc.tile_pool(name="ps", bufs=8, space="PSUM"))

            # ---- quant scale: [128,1] 1/qs ----
            tqs = consts.tile([P, 1], f32, tag="tqs")
            nc.sync.dma_start(tqs[:], qs_d.ap().partition_broadcast(P))
            tinv = consts.tile([P, 1], f32, tag="tinv")
            nc.vector.reciprocal(tinv[:], tqs[:])
            zero_t = consts.tile([P, 1], f32, tag="zero_t")
            nc.vector.memset(zero_t[:], 0.0)
            eps_t = consts.tile([P, 1], f32, tag="eps_t")
            nc.vector.memset(eps_t[:], LN_EPS)

            # ---- replicated per-channel params (general path only) ----
            # scale is folded into the decoded W planes during prep, so the
            # matmul already yields x @ (tern(W)*scale)^T; bias*scale is added
            # at evacuation; gamma/beta are applied in the epilogue.
            if not trivial_params:
                s_rep = consts.tile([P, DOUT], f32, tag="s_rep")
                nc.sync.dma_start(s_rep[:], scale_d.ap().partition_broadcast(P))
                bs_rep = consts.tile([P, DOUT], f32, tag="bs_rep")
                nc.sync.dma_start(bs_rep[:], bias_d.ap().partition_broadcast(P))
                nc.vector.tensor_tensor(bs_rep[:], bs_rep[:], s_rep[:], Alu.mult)
                g_rep = consts.tile([P, DOUT], f32, tag="g_rep")
                nc.sync.dma_start(g_rep[:], gam_d.ap().partition_broadcast(P))
                be_rep = consts.tile([P, DOUT], f32, tag="be_rep")
                nc.sync.dma_start(be_rep[:], bet_d.ap().partition_broadcast(P))

            # ---- W prep: decode base-27 packed W^T -> fp16 scratch ----
            # wp[d, j] = 27*w0 + 9*w1 + 3*w2 + w3 (each in {-1,0,1}) where
            # plane wi covers o-columns [i*1024, (i+1)*1024). Decoded with
            # round(r/base) via the magic-number trick; exact in f32.
            QW = DOUT // 4
            for db in range(DIN // P):
                wpt = wprep.tile([P, QW], i8, tag="wp", name=f"wp_{db}")
                nc.sync.dma_start(wpt[:], wp_d.ap()[db * P:(db + 1) * P, :])
                wf = wprep.tile([P, DOUT], f16, tag="wf", name=f"wf_{db}")
                resid = wprep.tile([P, QW], f32, tag="resid", name=f"re_{db}")
                nc.vector.tensor_copy(resid[:], wpt[:])
                for lvl, base in enumerate((27.0, 9.0, 3.0)):
                    q = wprep.tile([P, QW], f32, tag="q", name=f"q_{db}_{lvl}")
                    nc.vector.tensor_scalar(
                        q[:], resid[:], 1.0 / base, MAGIC, Alu.mult, Alu.add)
                    nc.vector.tensor_scalar(q[:], q[:], MAGIC, None, Alu.subtract)
                    wsl = wf[:, lvl * QW:(lvl + 1) * QW]
                    if trivial_params:
                        nc.vector.tensor_copy(wsl, q[:])
                    else:
                        qsc = wprep.tile([P, QW], f32, tag="qsc",
                                         name=f"qsc_{db}_{lvl}")
                        nc.vector.tensor_tensor(
                            qsc[:], q[:], s_rep[:, lvl * QW:(lvl + 1) * QW],
                            Alu.mult)
                        nc.vector.tensor_copy(wsl, qsc[:])
                    nc.vector.tensor_scalar(
                        q[:], q[:], -base, None, Alu.mult)
                    nc.vector.tensor_tensor(resid[:], resid[:], q[:], Alu.add)
                if trivial_params:
                    nc.vector.tensor_copy(wf[:, 3 * QW:], resid[:])
                else:
                    qsc = wprep.tile([P, QW], f32, tag="qsc",
                                     name=f"qsc_{db}_3")
                    nc.vector.tensor_tensor(
                        qsc[:], resid[:], s_rep[:, 3 * QW:], Alu.mult)
                    nc.vector.tensor_copy(wf[:, 3 * QW:], qsc[:])
                nc.sync.dma_start(wt_h.ap()[db * P:(db + 1) * P, :], wf[:])

            # ---- main loop over token blocks ----
            for blk in range(NBLK):
                t0 = blk * T_B
                # x^T for this block: [128 d, KT, T_B] fp16 via DMA transpose
                xt = xt_pool.tile([P, KT, T_B], f16, tag="xt")
                for k in range(KT):
                    nc.sync.dma_start(
                        xt[:, k, :],
                        x_d.ap()[t0:t0 + T_B, k * P:(k + 1) * P],
                        transpose=True)

                for op in range(NOP):
                    o0 = op * OPW
                    banks = []
                    for tt in range(NTT):
                        for os_ in range(NOS):
                            bank_t = pp.tile([P, 512], f32, tag="bank",
                                             name=f"bank_{blk}_{op}_{tt}_{os_}")
                            banks.append(bank_t)
                    # stream W^T fp16 quarter-slabs and accumulate
                    for k in range(KT):
                        ws = wst.tile([P, OPW], f16, tag="ws")
                        nc.sync.dma_start(
                            ws[:], wt_h.ap()[k * P:(k + 1) * P, o0:o0 + OPW])
                        for tt in range(NTT):
                            for os_ in range(NOS):
                                nc.tensor.matmul(
                                    banks[tt * NOS + os_][:],
                                    xt[:, k, tt * P:(tt + 1) * P],
                                    ws[:, os_ * 512:(os_ + 1) * 512],
                                    start=(k == 0), stop=(k == KT - 1))
                    # evacuate + stats
                    for tt in range(NTT):
                        if op == 0:
                            y = ypool.tile([P, DOUT], f32, tag="y")
                            sums = stat.tile([P, 8], f32, tag="sums")
                            sumsq = stat.tile([P, 8], f32, tag="sumsq")
                            if blk == 0 and tt == 0:
                                ylist, slist, qlist = [], [], []
                            ylist.append(y); slist.append(sums); qlist.append(sumsq)
                        y = ylist[tt]; sums = slist[tt]; sumsq = qlist[tt]
                        for os_ in range(NOS):
                            col = op * NOS + os_
                            zsl = y[:, o0 + os_ * 512: o0 + (os_ + 1) * 512]
                            bankap = banks[tt * NOS + os_][:]
                            if trivial_params:
                                nc.vector.tensor_scalar(
                                    zsl, bankap, 1.0, 0.0, Alu.mult, Alu.add,
                                    accum_out=sums[:, col:col + 1])
                            else:
                                # scale already folded into W; add bias*scale
                                nc.vector.tensor_tensor(
                                    zsl, bankap,
                                    bs_rep[:, o0 + os_ * 512: o0 + (os_ + 1) * 512],
                                    Alu.add)
                                zt = sq_pool.tile([P, 512], f32, tag="zt")
                                nc.vector.tensor_scalar(
                                    zt[:], zsl, 1.0, 0.0, Alu.mult, Alu.add,
                                    accum_out=sums[:, col:col + 1])
                            sq = sq_pool.tile([P, 512], f32, tag="sq")
                            nc.scalar.activation(
                                sq[:], zsl, Act.Square, bias=zero_t[:, 0:1],
                                accum_out=sumsq[:, col:col + 1])

                # ---- per-t-tile epilogue ----
                for tt in range(NTT):
                    y = ylist[tt]; sums = slist[tt]; sumsq = qlist[tt]
                    mu = stat.tile([P, 1], f32, tag="mu")
                    nc.vector.tensor_reduce(
                        out=mu[:], in_=sums[:], op=Alu.add,
                        axis=mybir.AxisListType.X)
                    nc.vector.tensor_scalar(mu[:], mu[:], 1.0 / DOUT, None, Alu.mult)
                    e2 = stat.tile([P, 1], f32, tag="e2")
                    nc.vector.tensor_reduce(
                        out=e2[:], in_=sumsq[:], op=Alu.add,
                        axis=mybir.AxisListType.X)
                    musq = stat.tile([P, 1], f32, tag="musq")
                    nc.vector.tensor_tensor(musq[:], mu[:], mu[:], Alu.mult)
                    var = stat.tile([P, 1], f32, tag="var")
                    nc.vector.tensor_scalar(
                        var[:], e2[:], 1.0 / DOUT, None, Alu.mult)
                    nc.vector.tensor_tensor(var[:], var[:], musq[:], Alu.subtract)
                    sd = stat.tile([P, 1], f32, tag="sd")
                    nc.scalar.activation(sd[:], var[:], Act.Sqrt, bias=eps_t[:, 0:1])
                    inv = stat.tile([P, 1], f32, tag="inv")
                    nc.vector.reciprocal(inv[:], sd[:])
                    # normalize in place: (z - mu) * inv
                    nc.vector.tensor_scalar(
                        y[:], y[:], mu[:, 0:1], inv[:, 0:1],
                        Alu.subtract, Alu.mult)
                    if not trivial_params:
                        nc.vector.tensor_tensor(y[:], y[:], g_rep[:], Alu.mult)
                        nc.vector.tensor_tensor(y[:], y[:], be_rep[:], Alu.add)
                    # tanh(y / qs)
                    nc.scalar.activation(y[:], y[:], Act.Tanh, bias=zero_t[:, 0:1], scale=tinv[:, 0:1])
                    # round(tanh*127) with round-half-even magic, to int8
                    nc.vector.tensor_scalar(
                        y[:], y[:], 127.0, MAGIC, Alu.mult, Alu.add)
                    oi = opool.tile([P, DOUT], i8, tag="oi")
                    nc.vector.tensor_scalar(
                        oi[:], y[:], MAGIC, None, Alu.subtract)
                    nc.sync.dma_start(
                        out_d.ap()[blk * T_B + tt * P: blk * T_B + (tt + 1) * P, :],
                        oi[:])

    nc.compile()
    return nc


KD = 16   # sparse path: max active input columns (padded)
KO = 16   # sparse path: max active output columns (padded)


def _build_sparse():
    """Sparse fast path (trivial params, <=16 active rows/cols of tern(W)).

    Inputs: xat [KD, T] fp16 (active x columns, pre-transposed, zero-padded),
    ta [KD, KO] fp16 (active ternary block), qs. For every token the kernel
    emits the KO active-column outputs plus the shared inactive-column value
    ("base", from y=0), all through the same LN/tanh/round pipeline.
    """
    NTT = T // P      # 16 t-tiles

    nc = bacc.Bacc("TRN2", target_bir_lowering=False, debug=False)
    xat_d = nc.dram_tensor("xat", [KD, T], f32, kind="ExternalInput")
    ta_d = nc.dram_tensor("ta", [KD, KO], f32, kind="ExternalInput")
    qs_d = nc.dram_tensor("qs", [1], f32, kind="ExternalInput")
    oc_d = nc.dram_tensor("oc", [T, KO + 1], i8, kind="ExternalOutput")

    with tile.TileContext(nc) as tc:
        with ExitStack() as ctx:
            consts = ctx.enter_context(tc.tile_pool(name="consts", bufs=1))
            work = ctx.enter_context(tc.tile_pool(name="work", bufs=4))
            stat = ctx.enter_context(tc.tile_pool(name="stat", bufs=8))
            pp = ctx.enter_context(tc.tile_pool(name="ps", bufs=4, space="PSUM"))

            tqs = consts.tile([P, 1], f32, tag="tqs")
            nc.sync.dma_start(tqs[:], qs_d.ap().partition_broadcast(P))
            tinv = consts.tile([P, 1], f32, tag="tinv")
            nc.vector.reciprocal(tinv[:], tqs[:])
            zero_t = consts.tile([P, 1], f32, tag="zero_t")
            nc.vector.memset(zero_t[:], 0.0)
            eps_t = consts.tile([P, 1], f32, tag="eps_t")
            nc.vector.memset(eps_t[:], LN_EPS)

            xa = consts.tile([KD, T], f32, tag="xa")
            nc.sync.dma_start(xa[:], xat_d.ap())
            ta = consts.tile([KD, KO], f32, tag="ta")
            nc.sync.dma_start(ta[:], ta_d.ap())

            for tt in range(NTT):
                ps = pp.tile([P, KO], f32, tag="ps", name=f"ps_{tt}")
                nc.tensor.matmul(
                    ps[:], xa[:, tt * P:(tt + 1) * P], ta[:],
                    start=True, stop=True)
                y = work.tile([P, KO], f32, tag="y", name=f"y_{tt}")
                sums = stat.tile([P, 1], f32, tag="sums")
                nc.vector.tensor_scalar(
                    y[:], ps[:], 1.0, 0.0, Alu.mult, Alu.add,
                    accum_out=sums[:])
                sq = work.tile([P, KO], f32, tag="sq", name=f"sq_{tt}")
                sumsq = stat.tile([P, 1], f32, tag="sumsq")
                nc.scalar.activation(
                    sq[:], y[:], Act.Square, bias=zero_t[:, 0:1],
                    accum_out=sumsq[:])
                mu = stat.tile([P, 1], f32, tag="mu")
                nc.vector.tensor_scalar(mu[:], sums[:], 1.0 / DOUT, None, Alu.mult)
                e2 = stat.tile([P, 1], f32, tag="e2")
                nc.vector.tensor_scalar(e2[:], sumsq[:], 1.0 / DOUT, None, Alu.mult)
                musq = stat.tile([P, 1], f32, tag="musq")
                nc.vector.tensor_tensor(musq[:], mu[:], mu[:], Alu.mult)
                var = stat.tile([P, 1], f32, tag="var")
                nc.vector.tensor_tensor(var[:], e2[:], musq[:], Alu.subtract)
                sd = stat.tile([P, 1], f32, tag="sd")
                nc.scalar.activation(sd[:], var[:], Act.Sqrt, bias=eps_t[:, 0:1])
                inv = stat.tile([P, 1], f32, tag="inv")
                nc.vector.reciprocal(inv[:], sd[:])
                big = work.tile([P, KO + 1], f32, tag="big", name=f"big_{tt}")
                nc.vector.tensor_scalar(
                    big[:, 0:KO], y[:], mu[:, 0:1], inv[:, 0:1],
                    Alu.subtract, Alu.mult)
                nc.vector.tensor_scalar(
                    big[:, KO:KO + 1], zero_t[:, 0:1], mu[:, 0:1], inv[:, 0:1],
                    Alu.subtract, Alu.mult)
                nc.scalar.activation(
                    big[:], big[:], Act.Tanh, bias=zero_t[:, 0:1],
                    scale=tinv[:, 0:1])
                nc.vector.tensor_scalar(
                    big[:], big[:], 127.0, MAGIC, Alu.mult, Alu.add)
                oc = work.tile([P, KO + 1], i8, tag="oc", name=f"oc_{tt}")
                nc.vector.tensor_scalar(
                    oc[:], big[:], MAGIC, None, Alu.subtract)
                nc.sync.dma_start(
                    oc_d.ap()[tt * P:(tt + 1) * P, :], oc[:])

    nc.compile()
    return nc


# ---------------- host-side runner ----------------

_IN_SHARDED = {"x", "xat", "out", "oc"}   # axis-0 sharded; rest replicated


def _make_runner(nc):
    """jit(shard_map) runner: x/out sharded on axis 0, params replicated."""
    bass2jax.install_neuronx_cc_hook()

    partition_name = (
        nc.partition_id_tensor.name if nc.partition_id_tensor else None
    )
    in_names, out_names, out_avals = [], [], []
    for alloc in nc.m.functions[0].allocations:
        if not isinstance(alloc, mybir.MemoryLocationSet):
            continue
        name = alloc.memorylocations[0].name
        if alloc.kind == "ExternalInput":
            if name != partition_name:
                in_names.append(name)
        elif alloc.kind == "ExternalOutput":
            out_names.append(name)
            shape = tuple(alloc.tensor_shape)
            dtype = mybir.dt.np(alloc.dtype)
            out_avals.append(jax.core.ShapedArray(shape, dtype))
    n_params = len(in_names)
    n_outs = len(out_avals)
    all_names = in_names + out_names
    if partition_name is not None:
        all_names.append(partition_name)
    donate = tuple(range(n_params, n_params + n_outs))

    def _body(*args):
        operands = list(args)
        if partition_name is not None:
            operands.append(bass2jax.partition_id_tensor())
        outs = bass2jax._bass_exec_p.bind(
            *operands,
            out_avals=tuple(out_avals),
            in_names=tuple(all_names),
            out_names=tuple(out_names),
            lowering_input_output_aliases=(),
            sim_require_finite=True,
            sim_require_nnan=True,
            nc=nc,
        )
        return tuple(outs)

    devices = jax.devices()[:NCORES]
    mesh = Mesh(np.asarray(devices), ("core",))
    shard_sh = NamedSharding(mesh, PartitionSpec("core"))
    repl_sh = NamedSharding(mesh, PartitionSpec())
    spec_of = lambda name: (
        PartitionSpec("core") if name in _IN_SHARDED else PartitionSpec()
    )
    in_specs = tuple(spec_of(n) for n in in_names) + tuple(
        PartitionSpec("core") for _ in out_names
    )
    out_specs = tuple(PartitionSpec("core") for _ in out_names)
    sharded = jax.jit(
        shard_map(_body, mesh=mesh, in_specs=in_specs, out_specs=out_specs,
                  check_rep=False),
        donate_argnums=donate,
        keep_unused=True,
    )

    # output donation buffers, created on-device (never shipped over the tunnel)
    zero_factories = []
    for name, aval in zip(out_names, out_avals):
        gshape = (NCORES * aval.shape[0],) + aval.shape[1:]
        zero_factories.append(
            jax.jit(partial(jnp.zeros, gshape, aval.dtype),
                    out_shardings=shard_sh)
        )

    def place(inputs: dict, pool: ThreadPoolExecutor):
        """Async per-device placement: sharded rows for x, replicas for rest."""
        placed = {}
        futs = []

        def put_shard(name, arr):
            rows = arr.shape[0] // NCORES
            parts = [
                jax.device_put(arr[c * rows:(c + 1) * rows], devices[c])
                for c in range(NCORES)
            ]
            placed[name] = jax.make_array_from_single_device_arrays(
                arr.shape, shard_sh, parts)

        def put_repl(name, arr):
            parts = [jax.device_put(arr, d) for d in devices]
            placed[name] = jax.make_array_from_single_device_arrays(
                arr.shape, repl_sh, parts)

        for n in in_names:
            if n in _IN_SHARDED:
                futs.append(pool.submit(put_shard, n, inputs[n]))
            else:
                futs.append(pool.submit(put_repl, n, inputs[n]))
        for f in futs:
            f.result()
        return [placed[n] for n in in_names]

    def run(inputs: dict, pool: ThreadPoolExecutor):
        args = place(inputs, pool)
        zeros = [zf() for zf in zero_factories]
        outs = sharded(*args, *zeros)
        return {n: outs[i] for i, n in enumerate(out_names)}

    return run


def _fetch_convert(jax_out):
    """Pipelined d2h fetch of int8 shards + convert to f32/127 on host."""
    out = np.empty((NCORES * T, DOUT), dtype=np.float32)
    shards = sorted(jax_out.addressable_shards,
                    key=lambda s: s.index[0].start or 0)
    with ThreadPoolExecutor(max_workers=2) as pool:
        futs = [(s.index, pool.submit(np.asarray, s.data)) for s in shards]
        for index, fut in futs:
            oi = fut.result()
            np.multiply(oi, np.float32(1.0 / 127.0), dtype=np.float32,
                        out=out[index], casting="unsafe")
    return out


def _pack_weight(tern):
    """Transpose + base-27 pack of the ternary weight: [DIN, DOUT//4] int8.

    wp[d, j] = 27*wt[d, j] + 9*wt[d, 1024+j] + 3*wt[d, 2048+j] + wt[d, 3072+j]
    """
    wt = np.ascontiguousarray(tern.T)            # [DIN, DOUT]
    Q = DOUT // 4
    wp = 27 * wt[:, :Q] + 9 * wt[:, Q:2 * Q] + 3 * wt[:, 2 * Q:3 * Q] + wt[:, 3 * Q:]
    return np.ascontiguousarray(wp.astype(np.int8))


def _run_once(run, inputs, fetch_name, convert):
    with ThreadPoolExecutor(max_workers=4) as pool:
        outs = run(inputs, pool)
    return convert(outs[fetch_name])


def _dense(x, tern, bias, scale, ln_gamma, ln_beta, quant_scale, trivial):
    if trivial not in _CACHE:
        nc = _build(trivial)
        _CACHE[trivial] = (nc, _make_runner(nc))
    nc, run = _CACHE[trivial]

    xh = np.ascontiguousarray(x.reshape(NCORES * T, DIN)).astype(np.float16)
    inputs = {
        "x": xh,
        "wp": _pack_weight(tern),
        "bias": np.asarray(bias, dtype=np.float32),
        "scale": np.asarray(scale, dtype=np.float32),
        "gam": np.asarray(ln_gamma, dtype=np.float32),
        "bet": np.asarray(ln_beta, dtype=np.float32),
        "qs": np.asarray(quant_scale, dtype=np.float32),
    }
    try:
        out = _run_once(run, inputs, "out", _fetch_convert)
    except Exception:
        out = _run_once(run, inputs, "out", _fetch_convert)
    return out.reshape(B, S, DOUT)


def _sparse(x, tern, o_act, d_act, quant_scale):
    if "sparse" not in _CACHE:
        nc = _build_sparse()
        _CACHE["sparse"] = (nc, _make_runner(nc))
    nc, run = _CACHE["sparse"]

    kd, ko = len(d_act), len(o_act)
    # active x columns, per-core transposed to [KD, T], zero-padded
    xs = np.ascontiguousarray(x.reshape(NCORES * T, DIN)[:, d_act], dtype=np.float32)
    xat = np.zeros((NCORES, KD, T), dtype=np.float32)
    xat[:, :kd, :] = xs.reshape(NCORES, T, kd).transpose(0, 2, 1)
    ta = np.zeros((KD, KO), dtype=np.float32)
    ta[:kd, :ko] = tern[np.ix_(o_act, d_act)].T

    inputs = {
        "xat": xat.reshape(NCORES * KD, T),
        "ta": ta,
        "qs": np.asarray(quant_scale, dtype=np.float32),
    }

    def convert(jax_oc):
        oc = np.asarray(jax_oc)                      # [NCORES*T, KO+1] int8
        inv127 = np.float32(1.0 / 127.0)
        out = np.empty((NCORES * T, DOUT), dtype=np.float32)
        out[:] = (oc[:, KO] * inv127)[:, None]
        if ko:
            out[:, o_act] = oc[:, :ko] * inv127
        return out

    try:
        out = _run_once(run, inputs, "oc", convert)
    except Exception:
        out = _run_once(run, inputs, "oc", convert)
    return out.reshape(B, S, DOUT)


def kernel(x, weight, bias, scale, ln_gamma, ln_beta, quant_scale):
    trivial = (
        not np.any(bias) and not np.any(ln_beta)
        and np.all(scale == 1.0) and np.all(ln_gamma == 1.0)
    )
    x = np.asarray(x)
    w = np.asarray(weight, dtype=np.float32)
    tern = (w >= THRESH).astype(np.int8) - (w <= -THRESH).astype(np.int8)

    if trivial:
        nzo, nzd = np.nonzero(tern)
        o_act, d_act = np.unique(nzo), np.unique(nzd)
        if len(o_act) <= KO and len(d_act) <= KD:
            return _sparse(x, tern, o_act, d_act, quant_scale)
    return _dense(x, tern, bias, scale, ln_gamma, ln_beta, quant_scale, trivial)


# revision 17
# speedup vs baseline: 1.4535x; 1.4535x over previous
"""LowBitEncoder Trainium2 kernel.

y = LayerNorm((x @ tern(W).T + bias) * scale) -> tanh(y/qs) -> round-to-1/127 grid.

Wall-clock of kernel() is dominated by the axon host<->device tunnel
(~67 MB/s h2d, ~37 MB/s d2h), so the design minimizes shipped bytes.

Dense path (general):
  - x shipped as fp16 [16384, 4096] (128 MiB), batch-sharded over 8 cores,
    placed with per-device async device_put (parallel streams).
  - weight ternarized AND base-27 packed on host (4 values/byte,
    [DIN, DOUT/4] i8, 4 MiB), sent once per device via a replicated spec
    (no 8x host copies); decoded on-device to an fp16 W^T DRAM scratch
    with round(r/base) magic-number arithmetic (exact for ternary).
  - per 512-token block: DMA-transpose x^T tiles; stream W^T quarter-slabs;
    fp16 matmuls accumulate y[128t, 4096o] f32 in 8 PSUM banks; DVE evac
    (+row sums), ACT square (+row sumsq), LN normalize, ACT tanh(1/qs),
    round via magic trick, int8 out.
  - output returned as int8 grid indices (64 MiB); the d2h fetch is
    pipelined with the int8 -> f32/127 conversion shard by shard.
  - output donation buffers are created on-device (jnp.zeros), not shipped.

Sparse path (trivial LN params and <=16 active rows/cols of tern(W), which
BitNet-style U(-0.1,0.1) init with threshold 0.1 almost always yields):
ships only the active x columns pre-transposed (f32, ~1 MiB), computes the
active-column matmul + exact LayerNorm statistics + tanh/round on device,
and returns per-token active outputs plus the shared inactive-column value;
the host broadcast-fills the full [B,S,DOUT] f32 output.
"""
import numpy as np
from contextlib import ExitStack
from functools import partial
from concurrent.futures import ThreadPoolExecutor

import jax
import jax.numpy as jnp
from jax.sharding import Mesh, PartitionSpec, NamedSharding
from jax.experimental.shard_map import shard_map

from concourse import bacc
import concourse.tile as tile
import concourse.mybir as mybir
from concourse import bass2jax

B, S, DIN, DOUT = 8, 2048, 4096, 4096
P = 128
T = S                 # tokens per core (batch-sharded)
NCORES = 8
THRESH = 0.1
LN_EPS = 1e-5
MAGIC = 12582912.0    # 1.5 * 2**23: round-half-even for |v| < 2**22
f32, f16, i8 = mybir.dt.float32, mybir.dt.float16, mybir.dt.int8
Alu = mybir.AluOpType
Act = mybir.ActivationFunctionType

_CACHE = {}


def _build(trivial_params: bool):
    """Build the Bass program. trivial_params: bias==0, scale==1, gamma==1, beta==0."""
    T_B = 512 if trivial_params else 256       # tokens per block
    NBLK = T // T_B
    NTT = T_B // P                             # 4 t-tiles per block
    KT = DIN // P                              # 32 k-tiles
    NOP = 4                                    # o-quarter count
    OPW = DOUT // NOP                          # 1024 columns per quarter
    NOS = OPW // 512                           # 2 o-slices of 512 per quarter

    nc = bacc.Bacc("TRN2", target_bir_lowering=False, debug=False)
    x_d = nc.dram_tensor("x", [T, DIN], f16, kind="ExternalInput")
    wp_d = nc.dram_tensor("wp", [DIN, DOUT // 4], i8, kind="ExternalInput")
    bias_d = nc.dram_tensor("bias", [DOUT], f32, kind="ExternalInput")
    scale_d = nc.dram_tensor("scale", [DOUT], f32, kind="ExternalInput")
    gam_d = nc.dram_tensor("gam", [DOUT], f32, kind="ExternalInput")
    bet_d = nc.dram_tensor("bet", [DOUT], f32, kind="ExternalInput")
    qs_d = nc.dram_tensor("qs", [1], f32, kind="ExternalInput")
    out_d = nc.dram_tensor("out", [T, DOUT], i8, kind="ExternalOutput")
    wt_h = nc.dram_tensor("wt_h", [DIN, DOUT], f16)   # fp16 W^T scratch

    with tile.TileContext(nc) as tc:
        with ExitStack() as ctx:
            consts = ctx.enter_context(tc.tile_pool(name="consts", bufs=1))
            wprep = ctx.enter_context(tc.tile_pool(name="wprep", bufs=2))
            xt_pool = ctx.enter_context(tc.tile_pool(name="xt", bufs=2))
            wst = ctx.enter_context(tc.tile_pool(name="wst", bufs=3))
            ypool = ctx.enter_context(tc.tile_pool(name="y", bufs=NTT))
            opool = ctx.enter_context(tc.tile_pool(name="o", bufs=2))
            stat = ctx.enter_context(tc.tile_pool(name="stat", bufs=2 * NTT + 2))
            sq_pool = ctx.enter_context(tc.tile_pool(name="sq", bufs=2))
            pp = ctx.enter_context(tc.tile_pool(name="ps", bufs=8, space="PSUM"))

            # ---- quant scale: [128,1] 1/qs ----
            tqs = consts.tile([P, 1], f32, tag="tqs")
            nc.sync.dma_start(tqs[:], qs_d.ap().partition_broadcast(P))
            tinv = consts.tile([P, 1], f32, tag="tinv")
            nc.vector.reciprocal(tinv[:], tqs[:])
            zero_t = consts.tile([P, 1], f32, tag="zero_t")
            nc.vector.memset(zero_t[:], 0.0)
            eps_t = consts.tile([P, 1], f32, tag="eps_t")
            nc.vector.memset(eps_t[:], LN_EPS)

            # ---- replicated per-channel params (general path only) ----
            # scale is folded into the decoded W planes during prep, so the
            # matmul already yields x @ (tern(W)*scale)^T; bias*scale is added
            # at evacuation; gamma/beta are applied in the epilogue.
            if not trivial_params:
                s_rep = consts.tile([P, DOUT], f32, tag="s_rep")
                nc.sync.dma_start(s_rep[:], scale_d.ap().partition_broadcast(P))
                bs_rep = consts.tile([P, DOUT], f32, tag="bs_rep")
                nc.sync.dma_start(bs_rep[:], bias_d.ap().partition_broadcast(P))
                nc.vector.tensor_tensor(bs_rep[:], bs_rep[:], s_rep[:], Alu.mult)
                g_rep = consts.tile([P, DOUT], f32, tag="g_rep")
                nc.sync.dma_start(g_rep[:], gam_d.ap().partition_broadcast(P))
                be_rep = consts.tile([P, DOUT], f32, tag="be_rep")
                nc.sync.dma_start(be_rep[:], bet_d.ap().partition_broadcast(P))

            # ---- W prep: decode base-27 packed W^T -> fp16 scratch ----
            # wp[d, j] = 27*w0 + 9*w1 + 3*w2 + w3 (each in {-1,0,1}) where
            # plane wi covers o-columns [i*1024, (i+1)*1024). Decoded with
            # round(r/base) via the magic-number trick; exact in f32.
            QW = DOUT // 4
            for db in range(DIN // P):
                wpt = wprep.tile([P, QW], i8, tag="wp", name=f"wp_{db}")
                nc.sync.dma_start(wpt[:], wp_d.ap()[db * P:(db + 1) * P, :])
                wf = wprep.tile([P, DOUT], f16, tag="wf", name=f"wf_{db}")
                resid = wprep.tile([P, QW], f32, tag="resid", name=f"re_{db}")
                nc.vector.tensor_copy(resid[:], wpt[:])
                for lvl, base in enumerate((27.0, 9.0, 3.0)):
                    q = wprep.tile([P, QW], f32, tag="q", name=f"q_{db}_{lvl}")
                    nc.vector.tensor_scalar(
                        q[:], resid[:], 1.0 / base, MAGIC, Alu.mult, Alu.add)
                    nc.vector.tensor_scalar(q[:], q[:], MAGIC, None, Alu.subtract)
                    wsl = wf[:, lvl * QW:(lvl + 1) * QW]
                    if trivial_params:
                        nc.vector.tensor_copy(wsl, q[:])
                    else:
                        qsc = wprep.tile([P, QW], f32, tag="qsc",
                                         name=f"qsc_{db}_{lvl}")
                        nc.vector.tensor_tensor(
                            qsc[:], q[:], s_rep[:, lvl * QW:(lvl + 1) * QW],
                            Alu.mult)
                        nc.vector.tensor_copy(wsl, qsc[:])
                    nc.vector.tensor_scalar(
                        q[:], q[:], -base, None, Alu.mult)
                    nc.vector.tensor_tensor(resid[:], resid[:], q[:], Alu.add)
                if trivial_params:
                    nc.vector.tensor_copy(wf[:, 3 * QW:], resid[:])
                else:
                    qsc = wprep.tile([P, QW], f32, tag="qsc",
                                     name=f"qsc_{db}_3")
                    nc.vector.tensor_tensor(
                        qsc[:], resid[:], s_rep[:, 3 * QW:], Alu.mult)
                    nc.vector.tensor_copy(wf[:, 3 * QW:], qsc[:])
                nc.sync.dma_start(wt_h.ap()[db * P:(db + 1) * P, :], wf[:])

            # ---- main loop over token blocks ----
            for blk in range(NBLK):
                t0 = blk * T_B
                # x^T for this block: [128 d, KT, T_B] fp16 via DMA transpose
                xt = xt_pool.tile([P, KT, T_B], f16, tag="xt")
                for k in range(KT):
                    nc.sync.dma_start(
                        xt[:, k, :],
                        x_d.ap()[t0:t0 + T_B, k * P:(k + 1) * P],
                        transpose=True)

                for op in range(NOP):
                    o0 = op * OPW
                    banks = []
                    for tt in range(NTT):
                        for os_ in range(NOS):
                            bank_t = pp.tile([P, 512], f32, tag="bank",
                                             name=f"bank_{blk}_{op}_{tt}_{os_}")
                            banks.append(bank_t)
                    # stream W^T fp16 quarter-slabs and accumulate
                    for k in range(KT):
                        ws = wst.tile([P, OPW], f16, tag="ws")
                        nc.sync.dma_start(
                            ws[:], wt_h.ap()[k * P:(k + 1) * P, o0:o0 + OPW])
                        for tt in range(NTT):
                            for os_ in range(NOS):
                                nc.tensor.matmul(
                                    banks[tt * NOS + os_][:],
                                    xt[:, k, tt * P:(tt + 1) * P],
                                    ws[:, os_ * 512:(os_ + 1) * 512],
                                    start=(k == 0), stop=(k == KT - 1))
                    # evacuate + stats
                    for tt in range(NTT):
                        if op == 0:
                            y = ypool.tile([P, DOUT], f32, tag="y")
                            sums = stat.tile([P, 8], f32, tag="sums")
                            sumsq = stat.tile([P, 8], f32, tag="sumsq")
                            if blk == 0 and tt == 0:
                                ylist, slist, qlist = [], [], []
                            ylist.append(y); slist.append(sums); qlist.append(sumsq)
                        y = ylist[tt]; sums = slist[tt]; sumsq = qlist[tt]
                        for os_ in range(NOS):
                            col = op * NOS + os_
                            zsl = y[:, o0 + os_ * 512: o0 + (os_ + 1) * 512]
                            bankap = banks[tt * NOS + os_][:]
                            if trivial_params:
                                nc.vector.tensor_scalar(
                                    zsl, bankap, 1.0, 0.0, Alu.mult, Alu.add,
                                    accum_out=sums[:, col:col + 1])
                            else:
                                # scale already folded into W; add bias*scale
                                nc.vector.tensor_tensor(
                                    zsl, bankap,
                                    bs_rep[:, o0 + os_ * 512: o0 + (os_ + 1) * 512],
                                    Alu.add)
                                zt = sq_pool.tile([P, 512], f32, tag="zt")
                                nc.vector.tensor_scalar(
                                    zt[:], zsl, 1.0, 0.0, Alu.mult, Alu.add,
                                    accum_out=sums[:, col:col + 1])
                            sq = sq_pool.tile([P, 512], f32, tag="sq")
                            nc.scalar.activation(
                                sq[:], zsl, Act.Square, bias=zero_t[:, 0:1],
                                accum_out=sumsq[:, col:col + 1])

                # ---- per-t-tile epilogue ----
                for tt in range(NTT):
                    y = ylist[tt]; sums = slist[tt]; sumsq = qlist[tt]
                    mu = stat.tile([P, 1], f32, tag="mu")
                    nc.vector.tensor_reduce(
                        out=mu[:], in_=sums[:], op=Alu.add,
                        axis=mybir.AxisListType.X)
                    nc.vector.tensor_scalar(mu[:], mu[:], 1.0 / DOUT, None, Alu.mult)
                    e2 = stat.tile([P, 1], f32, tag="e2")
                    nc.vector.tensor_reduce(
                        out=e2[:], in_=sumsq[:], op=Alu.add,
                        axis=mybir.AxisListType.X)
                    musq = stat.tile([P, 1], f32, tag="musq")
                    nc.vector.tensor_tensor(musq[:], mu[:], mu[:], Alu.mult)
                    var = stat.tile([P, 1], f32, tag="var")
                    nc.vector.tensor_scalar(
                        var[:], e2[:], 1.0 / DOUT, None, Alu.mult)
                    nc.vector.tensor_tensor(var[:], var[:], musq[:], Alu.subtract)
                    sd = stat.tile([P, 1], f32, tag="sd")
                    nc.scalar.activation(sd[:], var[:], Act.Sqrt, bias=eps_t[:, 0:1])
                    inv = stat.tile([P, 1], f32, tag="inv")
                    nc.vector.reciprocal(inv[:], sd[:])
                    # normalize in place: (z - mu) * inv
                    nc.vector.tensor_scalar(
                        y[:], y[:], mu[:, 0:1], inv[:, 0:1],
                        Alu.subtract, Alu.mult)
                    if not trivial_params:
                        nc.vector.tensor_tensor(y[:], y[:], g_rep[:], Alu.mult)
                        nc.vector.tensor_tensor(y[:], y[:], be_rep[:], Alu.add)
                    # tanh(y / qs)
                    nc.scalar.activation(y[:], y[:], Act.Tanh, bias=zero_t[:, 0:1], scale=tinv[:, 0:1])
                    # round(tanh*127) with round-half-even magic, to int8
                    nc.vector.tensor_scalar(
                        y[:], y[:], 127.0, MAGIC, Alu.mult, Alu.add)
                    oi = opool.tile([P, DOUT], i8, tag="oi")
                    nc.vector.tensor_scalar(
                        oi[:], y[:], MAGIC, None, Alu.subtract)
                    nc.sync.dma_start(
                        out_d.ap()[blk * T_B + tt * P: blk * T_B + (tt + 1) * P, :],
                        oi[:])

    nc.compile()
    return nc


KD = 16   # sparse path: max active input columns (padded)
KO = 16   # sparse path: max active output columns (padded)


def _build_sparse():
    """Sparse fast path (trivial params, <=16 active rows/cols of tern(W)).

    Inputs: xat [KD, T] f32 (active x columns, pre-transposed, zero-padded),
    ta [KD, KO] f32 (active ternary block), qs. For every token the kernel
    emits the KO active-column outputs plus the shared inactive-column value
    ("base", from y=0), all through the same LN/tanh/round pipeline.
    """
    NTT = T // P      # 16 t-tiles

    nc = bacc.Bacc("TRN2", target_bir_lowering=False, debug=False)
    xat_d = nc.dram_tensor("xat", [KD, T], f32, kind="ExternalInput")
    ta_d = nc.dram_tensor("ta", [KD, KO], f32, kind="ExternalInput")
    qs_d = nc.dram_tensor("qs", [1], f32, kind="ExternalInput")
    oc_d = nc.dram_tensor("oc", [T, KO + 1], i8, kind="ExternalOutput")

    with tile.TileContext(nc) as tc:
        with ExitStack() as ctx:
            consts = ctx.enter_context(tc.tile_pool(name="consts", bufs=1))
            work = ctx.enter_context(tc.tile_pool(name="work", bufs=4))
            stat = ctx.enter_context(tc.tile_pool(name="stat", bufs=8))
            pp = ctx.enter_context(tc.tile_pool(name="ps", bufs=4, space="PSUM"))

            tqs = consts.tile([P, 1], f32, tag="tqs")
            nc.sync.dma_start(tqs[:], qs_d.ap().partition_broadcast(P))
            tinv = consts.tile([P, 1], f32, tag="tinv")
            nc.vector.reciprocal(tinv[:], tqs[:])
            zero_t = consts.tile([P, 1], f32, tag="zero_t")
            nc.vector.memset(zero_t[:], 0.0)
            eps_t = consts.tile([P, 1], f32, tag="eps_t")
            nc.vector.memset(eps_t[:], LN_EPS)

            xa = consts.tile([KD, T], f32, tag="xa")
            nc.sync.dma_start(xa[:], xat_d.ap())
            ta = consts.tile([KD, KO], f32, tag="ta")
            nc.sync.dma_start(ta[:], ta_d.ap())

            for tt in range(NTT):
                ps = pp.tile([P, KO], f32, tag="ps", name=f"ps_{tt}")
                nc.tensor.matmul(
                    ps[:], xa[:, tt * P:(tt + 1) * P], ta[:],
                    start=True, stop=True)
                y = work.tile([P, KO], f32, tag="y", name=f"y_{tt}")
                sums = stat.tile([P, 1], f32, tag="sums")
                nc.vector.tensor_scalar(
                    y[:], ps[:], 1.0, 0.0, Alu.mult, Alu.add,
                    accum_out=sums[:])
                sq = work.tile([P, KO], f32, tag="sq", name=f"sq_{tt}")
                sumsq = stat.tile([P, 1], f32, tag="sumsq")
                nc.scalar.activation(
                    sq[:], y[:], Act.Square, bias=zero_t[:, 0:1],
                    accum_out=sumsq[:])
                mu = stat.tile([P, 1], f32, tag="mu")
                nc.vector.tensor_scalar(mu[:], sums[:], 1.0 / DOUT, None, Alu.mult)
                e2 = stat.tile([P, 1], f32, tag="e2")
                nc.vector.tensor_scalar(e2[:], sumsq[:], 1.0 / DOUT, None, Alu.mult)
                musq = stat.tile([P, 1], f32, tag="musq")
                nc.vector.tensor_tensor(musq[:], mu[:], mu[:], Alu.mult)
                var = stat.tile([P, 1], f32, tag="var")
                nc.vector.tensor_tensor(var[:], e2[:], musq[:], Alu.subtract)
                sd = stat.tile([P, 1], f32, tag="sd")
                nc.scalar.activation(sd[:], var[:], Act.Sqrt, bias=eps_t[:, 0:1])
                inv = stat.tile([P, 1], f32, tag="inv")
                nc.vector.reciprocal(inv[:], sd[:])
                big = work.tile([P, KO + 1], f32, tag="big", name=f"big_{tt}")
                nc.vector.tensor_scalar(
                    big[:, 0:KO], y[:], mu[:, 0:1], inv[:, 0:1],
                    Alu.subtract, Alu.mult)
                nc.vector.tensor_scalar(
                    big[:, KO:KO + 1], zero_t[:, 0:1], mu[:, 0:1], inv[:, 0:1],
                    Alu.subtract, Alu.mult)
                nc.scalar.activation(
                    big[:], big[:], Act.Tanh, bias=zero_t[:, 0:1],
                    scale=tinv[:, 0:1])
                nc.vector.tensor_scalar(
                    big[:], big[:], 127.0, MAGIC, Alu.mult, Alu.add)
                oc = work.tile([P, KO + 1], i8, tag="oc", name=f"oc_{tt}")
                nc.vector.tensor_scalar(
                    oc[:], big[:], MAGIC, None, Alu.subtract)
                nc.sync.dma_start(
                    oc_d.ap()[tt * P:(tt + 1) * P, :], oc[:])

    nc.compile()
    return nc


# ---------------- host-side runner ----------------

_IN_SHARDED = {"x", "xat", "out", "oc"}   # axis-0 sharded; rest replicated


def _make_runner(nc):
    """jit(shard_map) runner: x/out sharded on axis 0, params replicated."""
    bass2jax.install_neuronx_cc_hook()

    partition_name = (
        nc.partition_id_tensor.name if nc.partition_id_tensor else None
    )
    in_names, out_names, out_avals = [], [], []
    for alloc in nc.m.functions[0].allocations:
        if not isinstance(alloc, mybir.MemoryLocationSet):
            continue
        name = alloc.memorylocations[0].name
        if alloc.kind == "ExternalInput":
            if name != partition_name:
                in_names.append(name)
        elif alloc.kind == "ExternalOutput":
            out_names.append(name)
            shape = tuple(alloc.tensor_shape)
            dtype = mybir.dt.np(alloc.dtype)
            out_avals.append(jax.core.ShapedArray(shape, dtype))
    n_params = len(in_names)
    n_outs = len(out_avals)
    all_names = in_names + out_names
    if partition_name is not None:
        all_names.append(partition_name)
    donate = tuple(range(n_params, n_params + n_outs))

    def _body(*args):
        operands = list(args)
        if partition_name is not None:
            operands.append(bass2jax.partition_id_tensor())
        outs = bass2jax._bass_exec_p.bind(
            *operands,
            out_avals=tuple(out_avals),
            in_names=tuple(all_names),
            out_names=tuple(out_names),
            lowering_input_output_aliases=(),
            sim_require_finite=True,
            sim_require_nnan=True,
            nc=nc,
        )
        return tuple(outs)

    devices = jax.devices()[:NCORES]
    mesh = Mesh(np.asarray(devices), ("core",))
    shard_sh = NamedSharding(mesh, PartitionSpec("core"))
    repl_sh = NamedSharding(mesh, PartitionSpec())
    spec_of = lambda name: (
        PartitionSpec("core") if name in _IN_SHARDED else PartitionSpec()
    )
    in_specs = tuple(spec_of(n) for n in in_names) + tuple(
        PartitionSpec("core") for _ in out_names
    )
    out_specs = tuple(PartitionSpec("core") for _ in out_names)
    sharded = jax.jit(
        shard_map(_body, mesh=mesh, in_specs=in_specs, out_specs=out_specs,
                  check_rep=False),
        donate_argnums=donate,
        keep_unused=True,
    )

    # output donation buffers, created on-device (never shipped over the tunnel)
    zero_factories = []
    for name, aval in zip(out_names, out_avals):
        gshape = (NCORES * aval.shape[0],) + aval.shape[1:]
        zero_factories.append(
            jax.jit(partial(jnp.zeros, gshape, aval.dtype),
                    out_shardings=shard_sh)
        )

    def place(inputs: dict, pool: ThreadPoolExecutor):
        """Async per-device placement: sharded rows for x, replicas for rest."""
        placed = {}
        futs = []

        def put_shard(name, arr):
            rows = arr.shape[0] // NCORES
            parts = [
                jax.device_put(arr[c * rows:(c + 1) * rows], devices[c])
                for c in range(NCORES)
            ]
            placed[name] = jax.make_array_from_single_device_arrays(
                arr.shape, shard_sh, parts)

        def put_repl(name, arr):
            parts = [jax.device_put(arr, d) for d in devices]
            placed[name] = jax.make_array_from_single_device_arrays(
                arr.shape, repl_sh, parts)

        for n in in_names:
            if n in _IN_SHARDED:
                futs.append(pool.submit(put_shard, n, inputs[n]))
            else:
                futs.append(pool.submit(put_repl, n, inputs[n]))
        for f in futs:
            f.result()
        return [placed[n] for n in in_names]

    def run(inputs: dict, pool: ThreadPoolExecutor):
        args = place(inputs, pool)
        zeros = [zf() for zf in zero_factories]
        outs = sharded(*args, *zeros)
        return {n: outs[i] for i, n in enumerate(out_names)}

    return run


def _fetch_convert(jax_out):
    """Pipelined d2h fetch of int8 shards + convert to f32/127 on host."""
    out = np.empty((NCORES * T, DOUT), dtype=np.float32)
    shards = sorted(jax_out.addressable_shards,
                    key=lambda s: s.index[0].start or 0)
    with ThreadPoolExecutor(max_workers=2) as pool:
        futs = [(s.index, pool.submit(np.asarray, s.data)) for s in shards]
        for index, fut in futs:
            oi = fut.result()
            np.multiply(oi, np.float32(1.0 / 127.0), dtype=np.float32,
                        out=out[index], casting="unsafe")
    return out


def _pack_weight(tern):
    """Transpose + base-27 pack of the ternary weight: [DIN, DOUT//4] int8.

    wp[d, j] = 27*wt[d, j] + 9*wt[d, 1024+j] + 3*wt[d, 2048+j] + wt[d, 3072+j]
    """
    wt = np.ascontiguousarray(tern.T)            # [DIN, DOUT]
    Q = DOUT // 4
    wp = 27 * wt[:, :Q] + 9 * wt[:, Q:2 * Q] + 3 * wt[:, 2 * Q:3 * Q] + wt[:, 3 * Q:]
    return np.ascontiguousarray(wp.astype(np.int8))


def _run_once(run, inputs, fetch_name, convert):
    with ThreadPoolExecutor(max_workers=4) as pool:
        outs = run(inputs, pool)
    return convert(outs[fetch_name])


def _dense(x, tern, bias, scale, ln_gamma, ln_beta, quant_scale, trivial):
    if trivial not in _CACHE:
        nc = _build(trivial)
        _CACHE[trivial] = (nc, _make_runner(nc))
    nc, run = _CACHE[trivial]

    xh = np.ascontiguousarray(x.reshape(NCORES * T, DIN)).astype(np.float16)
    inputs = {
        "x": xh,
        "wp": _pack_weight(tern),
        "bias": np.asarray(bias, dtype=np.float32),
        "scale": np.asarray(scale, dtype=np.float32),
        "gam": np.asarray(ln_gamma, dtype=np.float32),
        "bet": np.asarray(ln_beta, dtype=np.float32),
        "qs": np.asarray(quant_scale, dtype=np.float32),
    }
    try:
        out = _run_once(run, inputs, "out", _fetch_convert)
    except Exception:
        out = _run_once(run, inputs, "out", _fetch_convert)
    return out.reshape(B, S, DOUT)


def _sparse(x, tern, o_act, d_act, quant_scale):
    if "sparse" not in _CACHE:
        nc = _build_sparse()
        _CACHE["sparse"] = (nc, _make_runner(nc))
    nc, run = _CACHE["sparse"]

    kd, ko = len(d_act), len(o_act)
    # active x columns, per-core transposed to [KD, T], zero-padded
    xs = np.ascontiguousarray(x.reshape(NCORES * T, DIN)[:, d_act], dtype=np.float32)
    xat = np.zeros((NCORES, KD, T), dtype=np.float32)
    xat[:, :kd, :] = xs.reshape(NCORES, T, kd).transpose(0, 2, 1)
    ta = np.zeros((KD, KO), dtype=np.float32)
    ta[:kd, :ko] = tern[np.ix_(o_act, d_act)].T

    inputs = {
        "xat": xat.reshape(NCORES * KD, T),
        "ta": ta,
        "qs": np.asarray(quant_scale, dtype=np.float32),
    }

    def convert(jax_oc):
        oc = np.asarray(jax_oc)                      # [NCORES*T, KO+1] int8
        inv127 = np.float32(1.0 / 127.0)
        out = np.empty((NCORES * T, DOUT), dtype=np.float32)
        out[:] = (oc[:, KO] * inv127)[:, None]
        if ko:
            out[:, o_act] = oc[:, :ko] * inv127
        return out

    try:
        out = _run_once(run, inputs, "oc", convert)
    except Exception:
        out = _run_once(run, inputs, "oc", convert)
    return out.reshape(B, S, DOUT)


def kernel(x, weight, bias, scale, ln_gamma, ln_beta, quant_scale):
    trivial = (
        not np.any(bias) and not np.any(ln_beta)
        and np.all(scale == 1.0) and np.all(ln_gamma == 1.0)
    )
    x = np.asarray(x)
    w = np.asarray(weight, dtype=np.float32)
    tern = (w >= THRESH).astype(np.int8) - (w <= -THRESH).astype(np.int8)

    if trivial:
        nzo, nzd = np.nonzero(tern)
        o_act, d_act = np.unique(nzo), np.unique(nzd)
        if len(o_act) <= KO and len(d_act) <= KD:
            return _sparse(x, tern, o_act, d_act, quant_scale)
    return _dense(x, tern, bias, scale, ln_gamma, ln_beta, quant_scale, trivial)
